# revision 2
# baseline (speedup 1.0000x reference)
"""AdaptivePrecisionKVCache Trainium2 kernel (8 NeuronCores, SPMD).

Reference computation (per the nn.Module):
    mask = |kv| > 0.01
    small bin (|kv| <= 0.01): quantize to 15 levels over [min_s, max_s]
    large bin (|kv| >  0.01): quantize to 255 levels over [min_l, max_l]
    out = dequantized values (bin-wise round-trip), input passed through
          where a bin is empty/degenerate (never happens for randn input).

Distribution: data-parallel over the heads axis (16 heads -> 2 per core).
The four bin statistics become a tiny AllReduce(max) of negated mins/maxes.

Per-core pipeline (shard = (2,2,8192,128) f32 = 16MB viewed as (128, 32768)):
  pass A (stream tiles): absx = ACT Abs(x); m = (absx <= T) as uint8 (kept in
     SBUF); z = x*m; per-partition min/max of z (small bin; zero-pollution is
     safe since min_s < 0 < max_s for this input) and of x (large bin = global
     extremes since both tails exist); partials -> cross-partition gather by
     DMA -> per-core stats -> AllReduce(max) -> global stats.
  coefficients (on device): a=levels/denom, c=-bmin*a, d=denom/levels, e=bmin
     per bin; broadcast to 128 partitions via a DRAM round-trip.
  pass B (re-stream x): q_b = int32(ACT Identity(a_b*x + c_b)) (convert rounds
     to nearest-even); deq_b = q_b*d_b + e_b (DVE tensor_scalar dual-op);
     out = deq_l overwritten with deq_s where mask -> DMA out.
"""
import sys

if '/opt/trn_rl_repo' not in sys.path:
    sys.path.insert(0, '/opt/trn_rl_repo')

import numpy as np

from concourse.bass import Bass
from concourse import mybir
from concourse.tile import TileContext
from concourse.bass_utils import run_bass_kernel_spmd

NCORES = 8
B, H, S, D = 2, 16, 8192, 128
H_PER = H // NCORES                      # 2 heads per core
SHARD_ELEMS = B * H_PER * S * D          # 4,194,304
P = 128
FD = SHARD_ELEMS // P                    # 32768 floats per partition
TILE_FD = 2048
NTILES = FD // TILE_FD                   # 16
THRESH = 0.01
BIG = 1e30

AF = mybir.ActivationFunctionType
ALU = mybir.AluOpType
AX = mybir.AxisListType
F32 = mybir.dt.float32
I32 = mybir.dt.int32
U8 = mybir.dt.uint8


def _split_sync_waits(nc, maxw=1):
    """Walrus in this toolchain accepts at most one semaphore wait per
    instruction; move excess waits onto extra Drain instructions."""
    for f in nc.m.functions:
        for bb in f.blocks:
            insts = list(bb.instructions)
            out = []
            changed = False
            for inst in insts:
                si = inst.sync_info
                if si is not None and si.on_wait and len(si.on_wait) > maxw:
                    waits = list(si.on_wait)
                    extra, keep = waits[:-maxw], waits[-maxw:]
                    k = 0
                    while extra:
                        chunk, extra = extra[:maxw], extra[maxw:]
                        nd = mybir.InstDrain(
                            name=f"{inst.name}-wsplit{k}", ins=[], outs=[])
                        nd.engine = inst.engine
                        nd.sync_info = mybir.SyncInfo(on_wait=chunk, on_update=[])
                        out.append(nd)
                        k += 1
                    inst.sync_info = mybir.SyncInfo(
                        on_wait=keep, on_update=list(si.on_update or []))
                    changed = True
                out.append(inst)
            if changed:
                bb.instructions = out


def _build():
    nc = Bass(trn_type="TRN2")
    x_in = nc.declare_dram_parameter("x", [P, FD], F32, isOutput=False)
    y_out = nc.declare_dram_parameter("y", [P, FD], F32, isOutput=True)

    cc_in = nc.dram_tensor("cc_in", [1, 4], F32)
    cc_out = nc.dram_tensor("cc_out", [1, 4], F32, addr_space="Shared")
    coef_dram = nc.dram_tensor("coef_scratch", [1, 8], F32)

    with TileContext(nc) as tc:
        with tc.tile_pool(name="mask", bufs=1) as mpool, \
             tc.tile_pool(name="xs", bufs=3) as xpool, \
             tc.tile_pool(name="scr", bufs=3) as spool, \
             tc.tile_pool(name="qs", bufs=4) as qpool, \
             tc.tile_pool(name="outs", bufs=3) as opool, \
             tc.tile_pool(name="stat", bufs=1) as stpool:

            masks = []
            for i in range(NTILES):
                masks.append(mpool.tile([P, TILE_FD], U8, tag=f"m{i}",
                                        name=f"mtile{i}"))

            # ---- pass A: reductions ----
            partz = stpool.tile([P, 2 * NTILES], F32, tag="partz")
            partx = stpool.tile([P, 2 * NTILES], F32, tag="partx")
            for i in range(NTILES):
                xt = xpool.tile([P, TILE_FD], F32, tag="xa")
                nc.sync.dma_start(out=xt[:, :],
                                  in_=x_in[:, i * TILE_FD:(i + 1) * TILE_FD])
                absx = spool.tile([P, TILE_FD], F32, tag="scra")
                nc.scalar.activation(absx[:, :], xt[:, :], AF.Abs,
                                     bias=0.0, scale=1.0)
                nc.vector.tensor_scalar(masks[i][:, :], absx[:, :], THRESH, None,
                                        op0=ALU.is_le)
                zt = spool.tile([P, TILE_FD], F32, tag="scra")
                nc.vector.tensor_tensor(out=zt[:, :], in0=xt[:, :],
                                        in1=masks[i][:, :], op=ALU.mult)
                nc.vector.tensor_reduce(partz[:, 2 * i:2 * i + 1], zt[:, :],
                                        axis=AX.X, op=ALU.min)
                nc.vector.tensor_reduce(partz[:, 2 * i + 1:2 * i + 2], zt[:, :],
                                        axis=AX.X, op=ALU.max)
                nc.vector.tensor_reduce(partx[:, 2 * i:2 * i + 1], xt[:, :],
                                        axis=AX.X, op=ALU.min)
                nc.vector.tensor_reduce(partx[:, 2 * i + 1:2 * i + 2], xt[:, :],
                                        axis=AX.X, op=ALU.max)

            # second-level reduce over tiles: strided views (stride 2)
            part4 = stpool.tile([P, 4], F32, tag="part4")
            nc.vector.tensor_reduce(part4[:, 0:1], partz[:, 0:2 * NTILES:2],
                                    axis=AX.X, op=ALU.min)
            nc.vector.tensor_reduce(part4[:, 1:2], partz[:, 1:2 * NTILES:2],
                                    axis=AX.X, op=ALU.max)
            nc.vector.tensor_reduce(part4[:, 2:3], partx[:, 0:2 * NTILES:2],
                                    axis=AX.X, op=ALU.min)
            nc.vector.tensor_reduce(part4[:, 3:4], partx[:, 1:2 * NTILES:2],
                                    axis=AX.X, op=ALU.max)
            # negate mins so every channel is a max-reduce
            nc.vector.tensor_scalar(part4[:, 0:1], part4[:, 0:1], -1.0, None,
                                    op0=ALU.mult)
            nc.vector.tensor_scalar(part4[:, 2:3], part4[:, 2:3], -1.0, None,
                                    op0=ALU.mult)

            # cross-partition gather (DMA) + reduce -> per-core stats (1,4)
            g = stpool.tile([1, 4 * P], F32, tag="g")
            for c in range(4):
                nc.sync.dma_start(out=g[0:1, c * P:(c + 1) * P],
                                  in_=part4[:, c:c + 1])
            stats = stpool.tile([1, 4], F32, tag="stats")
            for c in range(4):
                nc.vector.tensor_reduce(stats[0:1, c:c + 1],
                                        g[0:1, c * P:(c + 1) * P],
                                        axis=AX.X, op=ALU.max)

            # AllReduce(max) of [-min_s, max_s, -min_l, max_l]
            nc.sync.dma_start(out=cc_in[0:1, :], in_=stats[0:1, :])
            nc.gpsimd.collective_compute(
                "AllReduce", ALU.max,
                replica_groups=[list(range(NCORES))],
                ins=[cc_in.ap().opt()],
                outs=[cc_out.ap().opt()],
            )
            gstats = stpool.tile([1, 4], F32, tag="gstats")
            nc.sync.dma_start(out=gstats[0:1, :], in_=cc_out[0:1, :])

            # ---- coefficients ----
            # gstats = [-bmin_s, bmax_s, -bmin_l, bmax_l]
            # coef (1,8) = [a_s, c_s, d_s, e_s, a_l, c_l, d_l, e_l]
            coef = stpool.tile([1, 8], F32, tag="coef")
            den = stpool.tile([1, 4], F32, tag="den")
            nc.vector.tensor_tensor(out=den[0:1, 0:2], in0=gstats[0:1, 1:4:2],
                                    in1=gstats[0:1, 0:3:2], op=ALU.add)
            nc.vector.reciprocal(den[0:1, 2:4], den[0:1, 0:2])
            nc.vector.tensor_scalar(coef[0:1, 0:1], den[0:1, 2:3], 15.0, None,
                                    op0=ALU.mult)
            nc.vector.tensor_scalar(coef[0:1, 4:5], den[0:1, 3:4], 255.0, None,
                                    op0=ALU.mult)
            nc.vector.tensor_tensor(out=coef[0:1, 1:2], in0=gstats[0:1, 0:1],
                                    in1=coef[0:1, 0:1], op=ALU.mult)
            nc.vector.tensor_tensor(out=coef[0:1, 5:6], in0=gstats[0:1, 2:3],
                                    in1=coef[0:1, 4:5], op=ALU.mult)
            nc.vector.tensor_scalar(coef[0:1, 2:3], den[0:1, 0:1], 1.0 / 15.0,
                                    None, op0=ALU.mult)
            nc.vector.tensor_scalar(coef[0:1, 6:7], den[0:1, 1:2], 1.0 / 255.0,
                                    None, op0=ALU.mult)
            nc.vector.tensor_scalar(coef[0:1, 3:4], gstats[0:1, 0:1], -1.0,
                                    None, op0=ALU.mult)
            nc.vector.tensor_scalar(coef[0:1, 7:8], gstats[0:1, 2:3], -1.0,
                                    None, op0=ALU.mult)

            coefb = stpool.tile([P, 8], F32, tag="coefb")
            nc.sync.dma_start(out=coef_dram[0:1, :], in_=coef[0:1, :])
            nc.sync.dma_start(out=coefb[:, :],
                              in_=coef_dram.ap().to_broadcast((P, 8)))

            # ---- pass B: quantize-dequantize-select ----
            for i in range(NTILES):
                xt = xpool.tile([P, TILE_FD], F32, tag="xb")
                nc.sync.dma_start(out=xt[:, :],
                                  in_=x_in[:, i * TILE_FD:(i + 1) * TILE_FD])
                qs = qpool.tile([P, TILE_FD], I32, tag="q")
                ql = qpool.tile([P, TILE_FD], I32, tag="q")
                nc.scalar.activation(qs[:, :], xt[:, :], AF.Identity,
                                     bias=coefb[:, 1:2], scale=coefb[:, 0:1])
                nc.scalar.activation(ql[:, :], xt[:, :], AF.Identity,
                                     bias=coefb[:, 5:6], scale=coefb[:, 4:5])
                deq_s = spool.tile([P, TILE_FD], F32, tag="scrb")
                outt = opool.tile([P, TILE_FD], F32, tag="out")
                nc.vector.tensor_scalar(deq_s[:, :], qs[:, :], coefb[:, 2:3],
                                        coefb[:, 3:4], op0=ALU.mult, op1=ALU.add)
                nc.vector.tensor_scalar(outt[:, :], ql[:, :], coefb[:, 6:7],
                                        coefb[:, 7:8], op0=ALU.mult, op1=ALU.add)
                nc.vector.copy_predicated(outt[:, :], masks[i][:, :], deq_s[:, :])
                nc.sync.dma_start(out=y_out[:, i * TILE_FD:(i + 1) * TILE_FD],
                                  in_=outt[:, :])

    _split_sync_waits(nc)
    return nc


_NC_CACHE = {}


def _get_nc():
    if "nc" not in _NC_CACHE:
        _NC_CACHE["nc"] = _build()
    return _NC_CACHE["nc"]


def kernel(kv_cache: np.ndarray, _trace: bool = False) -> np.ndarray:
    kv = np.ascontiguousarray(kv_cache, dtype=np.float32)
    assert kv.shape == (B, H, S, D), kv.shape

    in_maps = []
    for i in range(NCORES):
        shard = np.ascontiguousarray(kv[:, i * H_PER:(i + 1) * H_PER])
        in_maps.append({"x": shard.reshape(P, FD)})

    nc = _get_nc()
    res = run_bass_kernel_spmd(nc, in_maps, core_ids=list(range(NCORES)),
                               trace=_trace)

    out = np.empty((B, H, S, D), dtype=np.float32)
    for i in range(NCORES):
        out[:, i * H_PER:(i + 1) * H_PER] = (
            res.results[i]["y"].reshape(B, H_PER, S, D))
    if _trace:
        kernel.last_exec_time_ns = res.exec_time_ns
        kernel.last_results = res
    return out


# revision 7
# speedup vs baseline: 1.3481x; 1.3481x over previous
"""AdaptivePrecisionKVCache Trainium2 kernel (8 NeuronCores, SPMD).

Reference computation (per the nn.Module):
    mask = |kv| > 0.01
    small bin (|kv| <= 0.01): quantize to 15 levels over [min_s, max_s]
    large bin (|kv| >  0.01): quantize to 255 levels over [min_l, max_l]
    out = dequantized values (bin-wise round-trip), input passed through
          where a bin is empty/degenerate (never happens for randn input).

Distribution: data-parallel over the heads axis (16 heads -> 2 per core).
The four bin statistics become a tiny AllReduce(max) of negated mins/maxes.

Per-core pipeline (shard = (2,2,8192,128) f32 = 16MB viewed as (128, 32768)):
  pass A (stream tiles): absx = ACT Abs(x); m = (absx <= T) as uint8 (kept in
     SBUF); z = x*m; per-partition min/max of z (small bin; zero-pollution is
     safe since min_s < 0 < max_s for this input) and of x (large bin = global
     extremes since both tails exist); partials -> cross-partition gather by
     DMA -> per-core stats -> AllReduce(max) -> global stats.
  coefficients (on device): a=levels/denom, c=-bmin*a, d=denom/levels, e=bmin
     per bin; broadcast to 128 partitions via a DRAM round-trip.
  pass B (re-stream x): q_b = int32(ACT Identity(a_b*x + c_b)) (convert rounds
     to nearest-even); deq_b = q_b*d_b + e_b (DVE tensor_scalar dual-op);
     out = deq_l overwritten with deq_s where mask -> DMA out.
"""
import sys

if '/opt/trn_rl_repo' not in sys.path:
    sys.path.insert(0, '/opt/trn_rl_repo')

import numpy as np

from concourse.bass import Bass
from concourse import mybir
from concourse.tile import TileContext
from concourse.bass_utils import run_bass_kernel_spmd

NCORES = 8
B, H, S, D = 2, 16, 8192, 128
H_PER = H // NCORES                      # 2 heads per core
SHARD_ELEMS = B * H_PER * S * D          # 4,194,304
P = 128
FD = SHARD_ELEMS // P                    # 32768 floats per partition
TILE_FD = 4096
NTILES = FD // TILE_FD                   # 16
THRESH = 0.01
BIG = 1e30

AF = mybir.ActivationFunctionType
ALU = mybir.AluOpType
AX = mybir.AxisListType
F32 = mybir.dt.float32
I32 = mybir.dt.int32
I16 = mybir.dt.int16
U8 = mybir.dt.uint8


def _split_sync_waits(nc, maxw=1):
    """Walrus in this toolchain accepts at most one semaphore wait per
    instruction; move excess waits onto extra Drain instructions."""
    for f in nc.m.functions:
        for bb in f.blocks:
            insts = list(bb.instructions)
            out = []
            changed = False
            for inst in insts:
                si = inst.sync_info
                if si is not None and si.on_wait and len(si.on_wait) > maxw:
                    waits = list(si.on_wait)
                    extra, keep = waits[:-maxw], waits[-maxw:]
                    k = 0
                    while extra:
                        chunk, extra = extra[:maxw], extra[maxw:]
                        nd = mybir.InstDrain(
                            name=f"{inst.name}-wsplit{k}", ins=[], outs=[])
                        nd.engine = inst.engine
                        nd.sync_info = mybir.SyncInfo(on_wait=chunk, on_update=[])
                        out.append(nd)
                        k += 1
                    inst.sync_info = mybir.SyncInfo(
                        on_wait=keep, on_update=list(si.on_update or []))
                    changed = True
                out.append(inst)
            if changed:
                bb.instructions = out


def _build():
    nc = Bass(trn_type="TRN2")
    x_in = nc.declare_dram_parameter("x", [P, FD], F32, isOutput=False)
    y_out = nc.declare_dram_parameter("y", [P, FD], F32, isOutput=True)

    cc_in = nc.dram_tensor("cc_in", [1, 4], F32)
    cc_out = nc.dram_tensor("cc_out", [1, 4], F32, addr_space="Shared")
    ccw_in = nc.dram_tensor("ccw_in", [1, 1], F32)
    ccw_out = nc.dram_tensor("ccw_out", [1, 1], F32, addr_space="Shared")
    coef_dram = nc.dram_tensor("coef_scratch", [1, 8], F32)

    with TileContext(nc) as tc:
        with tc.tile_pool(name="mask", bufs=1) as mpool, \
             tc.tile_pool(name="xs", bufs=2) as xpool, \
             tc.tile_pool(name="scr", bufs=2) as spool, \
             tc.tile_pool(name="qs", bufs=3) as qpool, \
             tc.tile_pool(name="outs", bufs=2) as opool, \
             tc.tile_pool(name="stat", bufs=1) as stpool:

            # warm-up collective: primes ncfw so the real AllReduce is fast.
            # No data deps -> overlaps pass A on the CC core.
            wt = stpool.tile([1, 1], F32, tag="warm")
            nc.vector.memset(wt[0:1, :], 0.0)
            nc.sync.dma_start(out=ccw_in[0:1, :], in_=wt[0:1, :])
            nc.gpsimd.collective_compute(
                "AllReduce", ALU.max,
                replica_groups=[list(range(NCORES))],
                ins=[ccw_in.ap().opt()],
                outs=[ccw_out.ap().opt()],
            )

            masks = []
            for i in range(NTILES):
                masks.append(mpool.tile([P, TILE_FD], U8, tag=f"m{i}",
                                        name=f"mtile{i}"))

            # ---- pass A: reductions ----
            partz = stpool.tile([P, 2 * NTILES], F32, tag="partz")
            partx = stpool.tile([P, 2 * NTILES], F32, tag="partx")
            for i in range(NTILES):
                xt = xpool.tile([P, TILE_FD], F32, tag="xa")
                nc.sync.dma_start(out=xt[:, :],
                                  in_=x_in[:, i * TILE_FD:(i + 1) * TILE_FD])
                absx = spool.tile([P, TILE_FD], F32, tag="scra")
                nc.scalar.activation(absx[:, :], xt[:, :], AF.Abs,
                                     bias=0.0, scale=1.0)
                nc.vector.tensor_scalar(masks[i][:, :], absx[:, :], THRESH, None,
                                        op0=ALU.is_le)
                zt = spool.tile([P, TILE_FD], F32, tag="scra")
                nc.vector.tensor_tensor(out=zt[:, :], in0=xt[:, :],
                                        in1=masks[i][:, :], op=ALU.mult)
                nc.vector.tensor_reduce(partz[:, 2 * i:2 * i + 1], zt[:, :],
                                        axis=AX.X, op=ALU.min)
                nc.vector.tensor_reduce(partz[:, 2 * i + 1:2 * i + 2], zt[:, :],
                                        axis=AX.X, op=ALU.max)
                nc.vector.tensor_reduce(partx[:, 2 * i:2 * i + 1], xt[:, :],
                                        axis=AX.X, op=ALU.min)
                nc.vector.tensor_reduce(partx[:, 2 * i + 1:2 * i + 2], xt[:, :],
                                        axis=AX.X, op=ALU.max)

            # second-level reduce over tiles: strided views (stride 2)
            part4 = stpool.tile([P, 4], F32, tag="part4")
            nc.vector.tensor_reduce(part4[:, 0:1], partz[:, 0:2 * NTILES:2],
                                    axis=AX.X, op=ALU.min)
            nc.vector.tensor_reduce(part4[:, 1:2], partz[:, 1:2 * NTILES:2],
                                    axis=AX.X, op=ALU.max)
            nc.vector.tensor_reduce(part4[:, 2:3], partx[:, 0:2 * NTILES:2],
                                    axis=AX.X, op=ALU.min)
            nc.vector.tensor_reduce(part4[:, 3:4], partx[:, 1:2 * NTILES:2],
                                    axis=AX.X, op=ALU.max)
            # negate mins so every channel is a max-reduce
            nc.vector.tensor_scalar(part4[:, 0:1], part4[:, 0:1], -1.0, None,
                                    op0=ALU.mult)
            nc.vector.tensor_scalar(part4[:, 2:3], part4[:, 2:3], -1.0, None,
                                    op0=ALU.mult)

            # cross-partition gather (DMA) + reduce -> per-core stats (1,4)
            g = stpool.tile([1, 4 * P], F32, tag="g")
            for c in range(4):
                nc.sync.dma_start(out=g[0:1, c * P:(c + 1) * P],
                                  in_=part4[:, c:c + 1])
            stats = stpool.tile([1, 4], F32, tag="stats")
            for c in range(4):
                nc.vector.tensor_reduce(stats[0:1, c:c + 1],
                                        g[0:1, c * P:(c + 1) * P],
                                        axis=AX.X, op=ALU.max)

            # AllReduce(max) of [-min_s, max_s, -min_l, max_l]
            nc.sync.dma_start(out=cc_in[0:1, :], in_=stats[0:1, :])
            nc.gpsimd.collective_compute(
                "AllReduce", ALU.max,
                replica_groups=[list(range(NCORES))],
                ins=[cc_in.ap().opt()],
                outs=[cc_out.ap().opt()],
            )
            gstats = stpool.tile([1, 4], F32, tag="gstats")
            nc.sync.dma_start(out=gstats[0:1, :], in_=cc_out[0:1, :])

            # ---- coefficients ----
            # gstats = [-bmin_s, bmax_s, -bmin_l, bmax_l]
            # coef (1,8) = [a_s, c_s, d_s, e_s, a_l, c_l, d_l, e_l]
            coef = stpool.tile([1, 8], F32, tag="coef")
            den = stpool.tile([1, 4], F32, tag="den")
            nc.vector.tensor_tensor(out=den[0:1, 0:2], in0=gstats[0:1, 1:4:2],
                                    in1=gstats[0:1, 0:3:2], op=ALU.add)
            nc.vector.reciprocal(den[0:1, 2:4], den[0:1, 0:2])
            nc.vector.tensor_scalar(coef[0:1, 0:1], den[0:1, 2:3], 15.0, None,
                                    op0=ALU.mult)
            nc.vector.tensor_scalar(coef[0:1, 4:5], den[0:1, 3:4], 255.0, None,
                                    op0=ALU.mult)
            nc.vector.tensor_tensor(out=coef[0:1, 1:2], in0=gstats[0:1, 0:1],
                                    in1=coef[0:1, 0:1], op=ALU.mult)
            nc.vector.tensor_tensor(out=coef[0:1, 5:6], in0=gstats[0:1, 2:3],
                                    in1=coef[0:1, 4:5], op=ALU.mult)
            nc.vector.tensor_scalar(coef[0:1, 2:3], den[0:1, 0:1], 1.0 / 15.0,
                                    None, op0=ALU.mult)
            nc.vector.tensor_scalar(coef[0:1, 6:7], den[0:1, 1:2], 1.0 / 255.0,
                                    None, op0=ALU.mult)
            nc.vector.tensor_scalar(coef[0:1, 3:4], gstats[0:1, 0:1], -1.0,
                                    None, op0=ALU.mult)
            nc.vector.tensor_scalar(coef[0:1, 7:8], gstats[0:1, 2:3], -1.0,
                                    None, op0=ALU.mult)

            coefb = stpool.tile([P, 8], F32, tag="coefb")
            nc.sync.dma_start(out=coef_dram[0:1, :], in_=coef[0:1, :])
            nc.sync.dma_start(out=coefb[:, :],
                              in_=coef_dram.ap().to_broadcast((P, 8)))

            # ---- pass B: quantize-dequantize-select ----
            for i in range(NTILES):
                xt = xpool.tile([P, TILE_FD], F32, tag="xa")
                nc.sync.dma_start(out=xt[:, :],
                                  in_=x_in[:, i * TILE_FD:(i + 1) * TILE_FD])
                qs = qpool.tile([P, TILE_FD], I16, tag="q")
                ql = qpool.tile([P, TILE_FD], I16, tag="q")
                nc.scalar.activation(qs[:, :], xt[:, :], AF.Identity,
                                     bias=coefb[:, 1:2], scale=coefb[:, 0:1])
                nc.scalar.activation(ql[:, :], xt[:, :], AF.Identity,
                                     bias=coefb[:, 5:6], scale=coefb[:, 4:5])
                deq_s = spool.tile([P, TILE_FD], F32, tag="scra")
                outt = opool.tile([P, TILE_FD], F32, tag="out")
                nc.vector.tensor_scalar(deq_s[:, :], qs[:, :], coefb[:, 2:3],
                                        coefb[:, 3:4], op0=ALU.mult, op1=ALU.add)
                nc.vector.tensor_scalar(outt[:, :], ql[:, :], coefb[:, 6:7],
                                        coefb[:, 7:8], op0=ALU.mult, op1=ALU.add)
                nc.vector.copy_predicated(outt[:, :], masks[i][:, :], deq_s[:, :])
                nc.sync.dma_start(out=y_out[:, i * TILE_FD:(i + 1) * TILE_FD],
                                  in_=outt[:, :])

    _split_sync_waits(nc)
    return nc


_NC_CACHE = {}


def _get_nc():
    if "nc" not in _NC_CACHE:
        _NC_CACHE["nc"] = _build()
    return _NC_CACHE["nc"]


def kernel(kv_cache: np.ndarray, _trace: bool = False) -> np.ndarray:
    kv = np.ascontiguousarray(kv_cache, dtype=np.float32)
    assert kv.shape == (B, H, S, D), kv.shape

    in_maps = []
    for i in range(NCORES):
        shard = np.ascontiguousarray(kv[:, i * H_PER:(i + 1) * H_PER])
        in_maps.append({"x": shard.reshape(P, FD)})

    nc = _get_nc()
    res = run_bass_kernel_spmd(nc, in_maps, core_ids=list(range(NCORES)),
                               trace=_trace)

    out = np.empty((B, H, S, D), dtype=np.float32)
    for i in range(NCORES):
        out[:, i * H_PER:(i + 1) * H_PER] = (
            res.results[i]["y"].reshape(B, H_PER, S, D))
    if _trace:
        kernel.last_exec_time_ns = res.exec_time_ns
        kernel.last_results = res
    return out
